# revision 14
# baseline (speedup 1.0000x reference)
"""Single-head attention (InterModalAttention) Bass kernel for 8 TRN2 cores.

Sharding: batch (4) x query-half (2) -> 8 cores; each core owns 1024 queries
of one batch element and attends over the full 2048-key sequence.

Algebraic reformulation (removes K/V projections and all PE transposes):
  scores[i,j] = q_i . k_j with q = x Wq^T + bq, k = x Wk^T + bk.
  The j-constant terms (q_i . bk) cancel in softmax, so with
      M  = Wq^T Wk        (host-precomputed, [d, d'])
      bu = bq  Wk         (host-precomputed, [d'])
      u  = x_q M + bu     (on-device "U projection", [i, d'])
  scoresT[j, i] = sum_d' x[j, d'] u[i, d']  -- lhsT = xT (raw input!).
  Output side:  out = attn v + bv = (attn x) Wv^T + bv:
      attnXT[d, i] = sum_j xN[j, d] attnT[j, i]   (attnT = exp(scoresT/32))
      out[i, e]    = sum_d attnXT[d, i] WvT[d, e] * inv_rowsum_i + bv[e]
  Row sums via ones-lhsT matmul on attnT; normalization folded into epilogue.

All matmul operands fp16 (PSUM accumulation fp32; verified rel err ~3.5e-4).
Per-core PE load ~410k rows vs ~616k for the direct q/k/v formulation.
"""
import sys
import numpy as np

for p in ("/opt/trn_rl_repo",):
    if p not in sys.path:
        sys.path.insert(0, p)

B, S, D = 4, 2048, 1024
NQ = 1024          # queries per core
NCORES = 8
P = 128
INV_SQRT_D = 1.0 / 32.0

_CACHE = {}


def build_nc():
    from contextlib import ExitStack
    import concourse.mybir as mybir
    import concourse.tile as tile
    from concourse import bacc

    F32 = mybir.dt.float32
    F16 = mybir.dt.float16
    F8 = mybir.dt.float8e4
    DR = mybir.MatmulPerfMode.DoubleRow
    AF = mybir.ActivationFunctionType

    nc = bacc.Bacc("TRN2", debug=False)

    xq16 = nc.dram_tensor("xq16", (D, NQ), F16, kind="ExternalInput")
    xT8 = nc.dram_tensor("xT8", (D, S), F8, kind="ExternalInput")
    xN = nc.dram_tensor("xN", (S, D), F16, kind="ExternalInput")
    m = nc.dram_tensor("m", (D, D), F16, kind="ExternalInput")
    wvT = nc.dram_tensor("wvT", (D, D), F16, kind="ExternalInput")
    bu = nc.dram_tensor("bu", (P, D // P), F32, kind="ExternalInput")
    bv16 = nc.dram_tensor("bv16", (D,), F16, kind="ExternalInput")
    out = nc.dram_tensor("out", (NQ, D), F32, kind="ExternalOutput")

    DT = D // P            # 8 d-tiles
    SB = S // P            # 16 j-tiles
    IG = NQ // 512         # 2 i-chunks
    EC = D // 512          # 2 e-chunks
    IB = 512 // P          # 4 i-subtiles per chunk

    with tile.TileContext(nc) as tc, ExitStack() as ctx:
        consts = ctx.enter_context(tc.tile_pool(name="consts", bufs=1))
        xt_pool = ctx.enter_context(tc.tile_pool(name="xt", bufs=1))
        xn_pool = ctx.enter_context(tc.tile_pool(name="xn", bufs=1))
        wv_pool = ctx.enter_context(tc.tile_pool(name="wv", bufs=1))
        u_pool = ctx.enter_context(tc.tile_pool(name="u", bufs=1))
        dram = ctx.enter_context(tc.tile_pool(name="dram", bufs=2, space="DRAM"))

        # sync + scalar are the hardware-DGE queues (fast); gpsimd DMAs go
        # through the software path -- give it only bulk with deadline slack
        _crit = [nc.sync, nc.scalar]
        _critc = [0]
        def dmac(out_ap, in_ap):
            e = _crit[_critc[0] % len(_crit)]
            _critc[0] += 1
            e.dma_start(out_ap, in_ap)

        _st = [nc.sync, nc.gpsimd]
        _stc = [0]
        def dma2(out_ap, in_ap):
            e = _st[_stc[0] % len(_st)]
            _stc[0] += 1
            e.dma_start(out_ap, in_ap)

        # ---- constants ----
        ones_col = consts.tile([P, 1], F16)
        nc.gpsimd.memset(ones_col[:], 1.0)
        ones_row = consts.tile([1, P], F16)
        nc.gpsimd.memset(ones_row[:], 1.0)
        bu_sb = consts.tile([P, DT], F32)
        bv_sb = consts.tile([1, D], F16)
        bv_bcast = consts.tile([P, D], F32)

        # fp8 copies of x^T for the DoubleRow scores matmul (dim1 = dt so
        # adjacent dt pairs sit in one AP); per-half tiles for tight deps
        xT8_lo = xt_pool.tile([P, DT, NQ], F8)
        xT8_hi = xt_pool.tile([P, DT, NQ], F8)
        # fp16 x^T of this core's queries, per-dt tiles (U projection input)
        xq_sb = []
        for dt in range(DT):
            t_ = xt_pool.tile([P, NQ], F16, tag=f"xq{dt}", name=f"xq{dt}")
            xq_sb.append(t_)
        xN_sb = xn_pool.tile([P, SB, D], F16)
        wv_sb = wv_pool.tile([P, DT, D], F16)
        uT_sb = []
        for g in range(IG):
            t_ = u_pool.tile([P, DT, 512], F8, tag=f"uT{g}", name=f"uT{g}")
            uT_sb.append(t_)

        # ---- Phase U: u = xq M + bu, streamed against the input DMAs ----
        # x layouts are j-rotated per core so the core's own 1024 queries are
        # always columns 0:1024 of xT (uniform SPMD program); attention is
        # invariant to the j-permutation because xN uses the same order.
        with tc.tile_pool(name="mp", bufs=1) as mp, \
             tc.tile_pool(name="ups", bufs=1, space="PSUM") as ups:
            m_sb = []
            for dt in range(DT):
                t_ = mp.tile([P, D], F16, tag=f"m{dt}", name=f"m{dt}")
                m_sb.append(t_)
            # load order: (m[dt] halves, xT_lo[dt, 0:512]) first so the
            # dt-outer U accumulation starts as soon as the first 256KB lands.
            # sync (slow software-DGE queue) carries the latest-needed lo
            # tiles plus the bulk that has deadline slack.
            for dt in range(DT):
                dmac(m_sb[dt][:, 0:512], m[dt * P:(dt + 1) * P, 0:512])
                if dt >= 5:
                    nc.sync.dma_start(xq_sb[dt][:, 0:512],
                                      xq16[dt * P:(dt + 1) * P, 0:512])
                else:
                    dmac(xq_sb[dt][:, 0:512], xq16[dt * P:(dt + 1) * P, 0:512])
                dmac(m_sb[dt][:, 512:1024], m[dt * P:(dt + 1) * P, 512:1024])
            nc.gpsimd.dma_start(bu_sb[:], bu[:, :])
            nc.gpsimd.dma_start(bv_sb[:], bv16[:].rearrange("(one d) -> one d",
                                                            one=1))
            for dt in range(DT):
                dmac(xq_sb[dt][:, 512:1024], xq16[dt * P:(dt + 1) * P, 512:1024])
            for dt in range(DT):
                dmac(xT8_lo[:, dt, :], xT8[dt * P:(dt + 1) * P, 0:1024])
            for dt in range(DT):
                dmac(xT8_hi[:, dt, :], xT8[dt * P:(dt + 1) * P, 1024:2048])
            # xN: half to the software queue (slack until beta), half hw
            for jb in range(SB):
                if jb % 2 == 0:
                    nc.gpsimd.dma_start(xN_sb[:, jb, :], xN[jb * P:(jb + 1) * P, :])
                else:
                    dmac(xN_sb[:, jb, :], xN[jb * P:(jb + 1) * P, :])
            for dt in range(DT):
                nc.gpsimd.dma_start(wv_sb[:, dt, :], wvT[dt * P:(dt + 1) * P, :])

            # g0: dt-outer streams against the input DMAs
            ups_t = [ups.tile([P, 512], F32, tag=f"ups{dc}", name=f"ups{dc}")
                     for dc in range(DT)]
            for dt in range(DT):
                for dc in range(DT):
                    nc.tensor.matmul(ups_t[dc][:], m_sb[dt][:, dc * P:(dc + 1) * P],
                                     xq_sb[dt][:, 0:512],
                                     start=(dt == 0), stop=(dt == DT - 1))
            for dc in range(DT):
                nc.vector.tensor_scalar_add(uT_sb[0][:, dc, :],
                                            ups_t[dc][:], bu_sb[:, dc:dc + 1])
            # g1: dc-outer so each copyback overlaps the next dc's matmuls
            for dc in range(DT):
                upt = ups.tile([P, 512], F32, tag=f"ups{dc}", name=f"upt{dc}")
                for dt in range(DT):
                    nc.tensor.matmul(upt[:], m_sb[dt][:, dc * P:(dc + 1) * P],
                                     xq_sb[dt][:, 512:1024],
                                     start=(dt == 0), stop=(dt == DT - 1))
                nc.vector.tensor_scalar_add(uT_sb[1][:, dc, :],
                                            upt[:], bu_sb[:, dc:dc + 1])

        # ---- attention-phase pools (8 PSUM banks total) ----
        ps = ctx.enter_context(tc.tile_pool(name="ps", bufs=3, space="PSUM"))
        axps = ctx.enter_context(tc.tile_pool(name="axps", bufs=2, space="PSUM"))
        outps = ctx.enter_context(tc.tile_pool(name="outps", bufs=2, space="PSUM"))
        rsps = ctx.enter_context(tc.tile_pool(name="rsps", bufs=1, space="PSUM"))
        attnp = ctx.enter_context(tc.tile_pool(name="attn", bufs=2))
        axp = ctx.enter_context(tc.tile_pool(name="ax", bufs=2))
        epip = ctx.enter_context(tc.tile_pool(name="epi", bufs=2))

        # bv broadcast [P, D] via ones_row.T @ bv (K=1 matmul)
        for ec in range(EC):
            pstmp = ps.tile([P, 512], F32, tag="ps")
            nc.tensor.matmul(pstmp[:], ones_row[:], bv_sb[:, ec * 512:(ec + 1) * 512],
                             start=True, stop=True)
            nc.vector.tensor_copy(bv_bcast[:, ec * 512:(ec + 1) * 512], pstmp[:])

        for g in range(IG):
            i0, i1 = g * 512, (g + 1) * 512
            attnT = attnp.tile([P, SB, 512], F16, tag="attnT")
            rsp = rsps.tile([1, 512], F32, tag="rs")
            # alpha: scoresT tiles + exp; rowsum matmul lags one jb so the
            # PE never waits on the ACT engine
            for jb in range(SB):
                scp = ps.tile([P, 512], F32, tag="ps")
                xsrc = xT8_lo if jb < 8 else xT8_hi
                j0 = (jb % 8) * P
                for t2 in range(DT // 2):
                    nc.tensor.matmul(scp[:], xsrc[:, 2 * t2:2 * t2 + 2, j0:j0 + P],
                                     uT_sb[g][:, 2 * t2:2 * t2 + 2, :],
                                     start=(t2 == 0), stop=(t2 == DT // 2 - 1),
                                     perf_mode=DR)
                nc.scalar.activation(attnT[:, jb, :], scp[:], AF.Exp, scale=INV_SQRT_D)
                if jb > 0:
                    nc.tensor.matmul(rsp[:], ones_col[:], attnT[:, jb - 1, :],
                                     start=(jb == 1), stop=False)
            # beta: attnXT[d, i] accumulation over j
            axT = axp.tile([P, DT, 512], F16, tag="axT")
            for dt in range(DT):
                axpt = axps.tile([P, 512], F32, tag="axps")
                for jb in range(SB):
                    nc.tensor.matmul(axpt[:], xN_sb[:, jb, dt * P:(dt + 1) * P],
                                     attnT[:, jb, :],
                                     start=(jb == 0), stop=(jb == SB - 1))
                if dt == 0:
                    nc.tensor.matmul(rsp[:], ones_col[:], attnT[:, SB - 1, :],
                                     start=False, stop=True)
                nc.vector.tensor_copy(axT[:, dt, :], axpt[:])
            # rowsum -> per-partition inverse (via DRAM bounce transpose)
            rs_row = epip.tile([1, 512], F32, tag="rs_row")
            nc.vector.tensor_copy(rs_row[:], rsp[:])
            rs_dram = dram.tile([512], F32, tag="rs_dram")
            dma2(rs_dram[:], rs_row[:])
            rs_col = epip.tile([P, IB], F32, tag="rs_col")
            dma2(rs_col[:], rs_dram[:].rearrange("(i p) -> p i", p=P))
            inv = epip.tile([P, IB], F32, tag="inv")
            nc.vector.reciprocal(inv[:], rs_col[:])
            # gamma: out[i, e] = attnXT.T @ WvT, normalized + bv
            for ib in range(IB):
                r0 = g * 512 + ib * P
                for ec in range(EC):
                    op = outps.tile([P, 512], F32, tag="outps")
                    for dt in range(DT):
                        nc.tensor.matmul(op[:], axT[:, dt, ib * P:(ib + 1) * P],
                                         wv_sb[:, dt, ec * 512:(ec + 1) * 512],
                                         start=(dt == 0), stop=(dt == DT - 1))
                    e0, e1 = ec * 512, (ec + 1) * 512
                    out_sb = epip.tile([P, 512], F32, tag="out_sb")
                    nc.vector.tensor_scalar_mul(out_sb[:], op[:], inv[:, ib:ib + 1])
                    nc.vector.tensor_add(out_sb[:], out_sb[:], bv_bcast[:, e0:e1])
                    if g == IG - 1 and ib == IB - 1:
                        (nc.sync if ec == 0 else nc.scalar).dma_start(
                            out[r0:r0 + P, e0:e0 + 256], out_sb[:, 0:256])
                        (nc.scalar if ec == 0 else nc.sync).dma_start(
                            out[r0:r0 + P, e0 + 256:e1], out_sb[:, 256:512])
                    else:
                        dma2(out[r0:r0 + P, e0:e1], out_sb[:])

    nc.compile()
    return nc


def make_in_maps(x, Wq, bq, Wk, bk, Wv, bv):
    x = np.asarray(x, np.float32)
    Wq = np.asarray(Wq, np.float32)
    Wk = np.asarray(Wk, np.float32)
    Wv = np.asarray(Wv, np.float32)
    m16 = np.ascontiguousarray((Wq.T @ Wk).astype(np.float16))
    bu32 = (np.asarray(bq, np.float32) @ Wk).astype(np.float32)
    bu32 = np.ascontiguousarray(bu32.reshape(-1, 128).T)  # [P, DT]
    wvT16 = np.ascontiguousarray(Wv.T.astype(np.float16))
    bv16 = np.ascontiguousarray(np.asarray(bv, np.float32).astype(np.float16))
    in_maps = []
    for c in range(NCORES):
        b, h = c // 2, c % 2
        # j-rotated so this core's queries are rows/cols 0:NQ (see build_nc)
        xb = np.roll(x[b], -h * NQ, axis=0)
        xb16 = xb.astype(np.float16)
        import ml_dtypes
        in_maps.append({
            "xq16": np.ascontiguousarray(xb16.T[:, 0:NQ]),
            "xT8": np.ascontiguousarray(xb.T.astype(ml_dtypes.float8_e4m3fn)),
            "xN": np.ascontiguousarray(xb16),
            "m": m16, "wvT": wvT16,
            "bu": bu32, "bv16": bv16,
        })
    return in_maps


def get_nc():
    if "nc" not in _CACHE:
        _CACHE["nc"] = build_nc()
    return _CACHE["nc"]


def kernel(x, Wq, bq, Wk, bk, Wv, bv):
    from concourse.bass_utils import run_bass_kernel_spmd
    nc = get_nc()
    in_maps = make_in_maps(x, Wq, bq, Wk, bk, Wv, bv)
    res = run_bass_kernel_spmd(nc, in_maps, core_ids=list(range(NCORES)))
    out = np.empty((B, S, D), np.float32)
    for c in range(NCORES):
        b, h = c // 2, c % 2
        out[b, h * NQ:(h + 1) * NQ] = res.results[c]["out"]
    return out


# revision 15
# speedup vs baseline: 1.0552x; 1.0552x over previous
"""Single-head attention (InterModalAttention) Bass kernel for 8 TRN2 cores.

Sharding: batch (4) x query-half (2) -> 8 cores; each core owns 1024 queries
of one batch element and attends over the full 2048-key sequence.

Algebraic reformulation (removes K/V projections and all PE transposes):
  scores[i,j] = q_i . k_j with q = x Wq^T + bq, k = x Wk^T + bk.
  The j-constant terms (q_i . bk) cancel in softmax, so with
      M  = Wq^T Wk        (host-precomputed, [d, d'])
      bu = bq  Wk         (host-precomputed, [d'])
      u  = x_q M + bu     (on-device "U projection", [i, d'])
  scoresT[j, i] = sum_d' x[j, d'] u[i, d']  -- lhsT = xT (raw input!).
  Output side:  out = attn v + bv = (attn x) Wv^T + bv:
      attnXT[d, i] = sum_j xN[j, d] attnT[j, i]   (attnT = exp(scoresT/32))
      out[i, e]    = sum_d attnXT[d, i] WvT[d, e] * inv_rowsum_i + bv[e]
  Row sums via ones-lhsT matmul on attnT; normalization folded into epilogue.

All matmul operands fp16 (PSUM accumulation fp32; verified rel err ~3.5e-4).
Per-core PE load ~410k rows vs ~616k for the direct q/k/v formulation.
"""
import sys
import numpy as np

for p in ("/opt/trn_rl_repo",):
    if p not in sys.path:
        sys.path.insert(0, p)

B, S, D = 4, 2048, 1024
NQ = 1024          # queries per core
NCORES = 8
P = 128
INV_SQRT_D = 1.0 / 32.0

_CACHE = {}


def build_nc():
    from contextlib import ExitStack
    import concourse.mybir as mybir
    import concourse.tile as tile
    from concourse import bacc

    F32 = mybir.dt.float32
    F16 = mybir.dt.float16
    F8 = mybir.dt.float8e4
    DR = mybir.MatmulPerfMode.DoubleRow
    AF = mybir.ActivationFunctionType

    nc = bacc.Bacc("TRN2", debug=False)

    xq16 = nc.dram_tensor("xq16", (D, NQ), F16, kind="ExternalInput")
    xT8 = nc.dram_tensor("xT8", (D, S), F8, kind="ExternalInput")
    xN = nc.dram_tensor("xN", (S, D), F16, kind="ExternalInput")
    m = nc.dram_tensor("m", (D, D), F16, kind="ExternalInput")
    wvT = nc.dram_tensor("wvT", (D, D), F16, kind="ExternalInput")
    bu = nc.dram_tensor("bu", (P, D // P), F32, kind="ExternalInput")
    bv16 = nc.dram_tensor("bv16", (D,), F16, kind="ExternalInput")
    out = nc.dram_tensor("out", (NQ, D), F32, kind="ExternalOutput")

    DT = D // P            # 8 d-tiles
    SB = S // P            # 16 j-tiles
    IG = NQ // 512         # 2 i-chunks
    EC = D // 512          # 2 e-chunks
    IB = 512 // P          # 4 i-subtiles per chunk

    with tile.TileContext(nc) as tc, ExitStack() as ctx:
        consts = ctx.enter_context(tc.tile_pool(name="consts", bufs=1))
        xt_pool = ctx.enter_context(tc.tile_pool(name="xt", bufs=1))
        xn_pool = ctx.enter_context(tc.tile_pool(name="xn", bufs=1))
        wv_pool = ctx.enter_context(tc.tile_pool(name="wv", bufs=1))
        u_pool = ctx.enter_context(tc.tile_pool(name="u", bufs=1))
        dram = ctx.enter_context(tc.tile_pool(name="dram", bufs=2, space="DRAM"))

        # gpsimd's software-DGE queue aggregates bigger packets (~2x the
        # throughput of each hardware queue) -- put the U-phase criticals
        # there, spread the rest over the sync/scalar hardware queues
        _crit = [nc.gpsimd, nc.sync, nc.scalar]
        _critc = [0]
        def dmac(out_ap, in_ap):
            e = _crit[_critc[0] % len(_crit)]
            _critc[0] += 1
            e.dma_start(out_ap, in_ap)

        _st = [nc.sync, nc.gpsimd]
        _stc = [0]
        def dma2(out_ap, in_ap):
            e = _st[_stc[0] % len(_st)]
            _stc[0] += 1
            e.dma_start(out_ap, in_ap)

        # ---- constants ----
        ones_col = consts.tile([P, 1], F16)
        nc.gpsimd.memset(ones_col[:], 1.0)
        ones_row = consts.tile([1, P], F16)
        nc.gpsimd.memset(ones_row[:], 1.0)
        bu_sb = consts.tile([P, DT], F32)
        bv_sb = consts.tile([1, D], F16)
        bv_bcast = consts.tile([P, D], F32)

        # fp8 copies of x^T for the DoubleRow scores matmul (dim1 = dt so
        # adjacent dt pairs sit in one AP); per-half tiles for tight deps
        xT8_lo = xt_pool.tile([P, DT, NQ], F8)
        xT8_hi = xt_pool.tile([P, DT, NQ], F8)
        # fp16 x^T of this core's queries, per-dt tiles (U projection input)
        xq_sb = []
        for dt in range(DT):
            t_ = xt_pool.tile([P, NQ], F16, tag=f"xq{dt}", name=f"xq{dt}")
            xq_sb.append(t_)
        xN_sb = xn_pool.tile([P, SB, D], F16)
        wv_sb = wv_pool.tile([P, DT, D], F16)
        uT_sb = []
        for g in range(IG):
            t_ = u_pool.tile([P, DT, 512], F8, tag=f"uT{g}", name=f"uT{g}")
            uT_sb.append(t_)

        # ---- Phase U: u = xq M + bu, streamed against the input DMAs ----
        # x layouts are j-rotated per core so the core's own 1024 queries are
        # always columns 0:1024 of xT (uniform SPMD program); attention is
        # invariant to the j-permutation because xN uses the same order.
        with tc.tile_pool(name="mp", bufs=1) as mp, \
             tc.tile_pool(name="ups", bufs=1, space="PSUM") as ups:
            m_sb = []
            for dt in range(DT):
                t_ = mp.tile([P, D], F16, tag=f"m{dt}", name=f"m{dt}")
                m_sb.append(t_)
            # load order: (m[dt] halves, xT_lo[dt, 0:512]) first so the
            # dt-outer U accumulation starts as soon as the first 256KB lands.
            # sync (slow software-DGE queue) carries the latest-needed lo
            # tiles plus the bulk that has deadline slack.
            # m on the fast software queue; xq on the two hardware queues
            for dt in range(DT):
                nc.gpsimd.dma_start(m_sb[dt][:, 0:512], m[dt * P:(dt + 1) * P, 0:512])
                (nc.sync if dt % 2 == 0 else nc.scalar).dma_start(
                    xq_sb[dt][:, 0:512], xq16[dt * P:(dt + 1) * P, 0:512])
                nc.gpsimd.dma_start(m_sb[dt][:, 512:1024],
                                    m[dt * P:(dt + 1) * P, 512:1024])
            for dt in range(DT):
                (nc.sync if dt % 2 == 0 else nc.scalar).dma_start(
                    xq_sb[dt][:, 512:1024], xq16[dt * P:(dt + 1) * P, 512:1024])
            nc.gpsimd.dma_start(bu_sb[:], bu[:, :])
            nc.gpsimd.dma_start(bv_sb[:], bv16[:].rearrange("(one d) -> one d",
                                                            one=1))
            for dt in range(DT):
                dmac(xT8_lo[:, dt, :], xT8[dt * P:(dt + 1) * P, 0:1024])
            for dt in range(DT):
                dmac(xT8_hi[:, dt, :], xT8[dt * P:(dt + 1) * P, 1024:2048])
            for jb in range(SB):
                dmac(xN_sb[:, jb, :], xN[jb * P:(jb + 1) * P, :])
            for dt in range(DT):
                dmac(wv_sb[:, dt, :], wvT[dt * P:(dt + 1) * P, :])

            # g0: dt-outer streams against the input DMAs
            ups_t = [ups.tile([P, 512], F32, tag=f"ups{dc}", name=f"ups{dc}")
                     for dc in range(DT)]
            for dt in range(DT):
                for dc in range(DT):
                    nc.tensor.matmul(ups_t[dc][:], m_sb[dt][:, dc * P:(dc + 1) * P],
                                     xq_sb[dt][:, 0:512],
                                     start=(dt == 0), stop=(dt == DT - 1))
            for dc in range(DT):
                nc.vector.tensor_scalar_add(uT_sb[0][:, dc, :],
                                            ups_t[dc][:], bu_sb[:, dc:dc + 1])
            # g1: dc-outer so each copyback overlaps the next dc's matmuls
            for dc in range(DT):
                upt = ups.tile([P, 512], F32, tag=f"ups{dc}", name=f"upt{dc}")
                for dt in range(DT):
                    nc.tensor.matmul(upt[:], m_sb[dt][:, dc * P:(dc + 1) * P],
                                     xq_sb[dt][:, 512:1024],
                                     start=(dt == 0), stop=(dt == DT - 1))
                nc.vector.tensor_scalar_add(uT_sb[1][:, dc, :],
                                            upt[:], bu_sb[:, dc:dc + 1])

        # ---- attention-phase pools (8 PSUM banks total) ----
        ps = ctx.enter_context(tc.tile_pool(name="ps", bufs=3, space="PSUM"))
        axps = ctx.enter_context(tc.tile_pool(name="axps", bufs=2, space="PSUM"))
        outps = ctx.enter_context(tc.tile_pool(name="outps", bufs=2, space="PSUM"))
        rsps = ctx.enter_context(tc.tile_pool(name="rsps", bufs=1, space="PSUM"))
        attnp = ctx.enter_context(tc.tile_pool(name="attn", bufs=2))
        axp = ctx.enter_context(tc.tile_pool(name="ax", bufs=2))
        epip = ctx.enter_context(tc.tile_pool(name="epi", bufs=2))

        # bv broadcast [P, D] via ones_row.T @ bv (K=1 matmul)
        for ec in range(EC):
            pstmp = ps.tile([P, 512], F32, tag="ps")
            nc.tensor.matmul(pstmp[:], ones_row[:], bv_sb[:, ec * 512:(ec + 1) * 512],
                             start=True, stop=True)
            nc.vector.tensor_copy(bv_bcast[:, ec * 512:(ec + 1) * 512], pstmp[:])

        for g in range(IG):
            i0, i1 = g * 512, (g + 1) * 512
            attnT = attnp.tile([P, SB, 512], F16, tag="attnT")
            rsp = rsps.tile([1, 512], F32, tag="rs")
            # alpha: scoresT tiles + exp; rowsum matmul lags one jb so the
            # PE never waits on the ACT engine
            for jb in range(SB):
                scp = ps.tile([P, 512], F32, tag="ps")
                xsrc = xT8_lo if jb < 8 else xT8_hi
                j0 = (jb % 8) * P
                for t2 in range(DT // 2):
                    nc.tensor.matmul(scp[:], xsrc[:, 2 * t2:2 * t2 + 2, j0:j0 + P],
                                     uT_sb[g][:, 2 * t2:2 * t2 + 2, :],
                                     start=(t2 == 0), stop=(t2 == DT // 2 - 1),
                                     perf_mode=DR)
                nc.scalar.activation(attnT[:, jb, :], scp[:], AF.Exp, scale=INV_SQRT_D)
                if jb > 0:
                    nc.tensor.matmul(rsp[:], ones_col[:], attnT[:, jb - 1, :],
                                     start=(jb == 1), stop=False)
            # beta: attnXT[d, i] accumulation over j
            axT = axp.tile([P, DT, 512], F16, tag="axT")
            for dt in range(DT):
                axpt = axps.tile([P, 512], F32, tag="axps")
                for jb in range(SB):
                    nc.tensor.matmul(axpt[:], xN_sb[:, jb, dt * P:(dt + 1) * P],
                                     attnT[:, jb, :],
                                     start=(jb == 0), stop=(jb == SB - 1))
                if dt == 0:
                    nc.tensor.matmul(rsp[:], ones_col[:], attnT[:, SB - 1, :],
                                     start=False, stop=True)
                nc.vector.tensor_copy(axT[:, dt, :], axpt[:])
            # rowsum -> per-partition inverse (via DRAM bounce transpose)
            rs_row = epip.tile([1, 512], F32, tag="rs_row")
            nc.vector.tensor_copy(rs_row[:], rsp[:])
            rs_dram = dram.tile([512], F32, tag="rs_dram")
            dma2(rs_dram[:], rs_row[:])
            rs_col = epip.tile([P, IB], F32, tag="rs_col")
            dma2(rs_col[:], rs_dram[:].rearrange("(i p) -> p i", p=P))
            inv = epip.tile([P, IB], F32, tag="inv")
            nc.vector.reciprocal(inv[:], rs_col[:])
            # gamma: out[i, e] = attnXT.T @ WvT, normalized + bv
            for ib in range(IB):
                r0 = g * 512 + ib * P
                for ec in range(EC):
                    op = outps.tile([P, 512], F32, tag="outps")
                    for dt in range(DT):
                        nc.tensor.matmul(op[:], axT[:, dt, ib * P:(ib + 1) * P],
                                         wv_sb[:, dt, ec * 512:(ec + 1) * 512],
                                         start=(dt == 0), stop=(dt == DT - 1))
                    e0, e1 = ec * 512, (ec + 1) * 512
                    out_sb = epip.tile([P, 512], F32, tag="out_sb")
                    nc.vector.tensor_scalar_mul(out_sb[:], op[:], inv[:, ib:ib + 1])
                    nc.vector.tensor_add(out_sb[:], out_sb[:], bv_bcast[:, e0:e1])
                    if g == IG - 1 and ib == IB - 1:
                        (nc.sync if ec == 0 else nc.scalar).dma_start(
                            out[r0:r0 + P, e0:e0 + 256], out_sb[:, 0:256])
                        (nc.scalar if ec == 0 else nc.sync).dma_start(
                            out[r0:r0 + P, e0 + 256:e1], out_sb[:, 256:512])
                    else:
                        dma2(out[r0:r0 + P, e0:e1], out_sb[:])

    nc.compile()
    return nc


def make_in_maps(x, Wq, bq, Wk, bk, Wv, bv):
    x = np.asarray(x, np.float32)
    Wq = np.asarray(Wq, np.float32)
    Wk = np.asarray(Wk, np.float32)
    Wv = np.asarray(Wv, np.float32)
    m16 = np.ascontiguousarray((Wq.T @ Wk).astype(np.float16))
    bu32 = (np.asarray(bq, np.float32) @ Wk).astype(np.float32)
    bu32 = np.ascontiguousarray(bu32.reshape(-1, 128).T)  # [P, DT]
    wvT16 = np.ascontiguousarray(Wv.T.astype(np.float16))
    bv16 = np.ascontiguousarray(np.asarray(bv, np.float32).astype(np.float16))
    in_maps = []
    for c in range(NCORES):
        b, h = c // 2, c % 2
        # j-rotated so this core's queries are rows/cols 0:NQ (see build_nc)
        xb = np.roll(x[b], -h * NQ, axis=0)
        xb16 = xb.astype(np.float16)
        import ml_dtypes
        in_maps.append({
            "xq16": np.ascontiguousarray(xb16.T[:, 0:NQ]),
            "xT8": np.ascontiguousarray(xb.T.astype(ml_dtypes.float8_e4m3fn)),
            "xN": np.ascontiguousarray(xb16),
            "m": m16, "wvT": wvT16,
            "bu": bu32, "bv16": bv16,
        })
    return in_maps


def get_nc():
    if "nc" not in _CACHE:
        _CACHE["nc"] = build_nc()
    return _CACHE["nc"]


def kernel(x, Wq, bq, Wk, bk, Wv, bv):
    from concourse.bass_utils import run_bass_kernel_spmd
    nc = get_nc()
    in_maps = make_in_maps(x, Wq, bq, Wk, bk, Wv, bv)
    res = run_bass_kernel_spmd(nc, in_maps, core_ids=list(range(NCORES)))
    out = np.empty((B, S, D), np.float32)
    for c in range(NCORES):
        b, h = c // 2, c % 2
        out[b, h * NQ:(h + 1) * NQ] = res.results[c]["out"]
    return out
